# revision 7
# baseline (speedup 1.0000x reference)
"""Trainium2 Bass kernel for nn_Attention_Module (sparse_attention).

Computation per batch b (x_b: [C=256, T=4096] fp32):
    energy = x_b @ x_b^T                      # (256, 256), K=4096
    attn   = softmax(rowmax(energy) - energy) # == exp(mu - e)/Z, mu = rowmin
    out    = gamma * (attn @ x_b) + x_b

Strategy (8 cores, pure data-parallel, 4 batches/core):
  - All HBM I/O in fp16 (24 MB/core): x host-cast/staged into xt
    [P, KT, C] (t on partitions, energy matmul) and xn [P, 2, T] (c on
    partitions, second matmul); fp16 output host-upcast afterward.
    Staged layouts make every DMA partition line >= 8 KB contiguous.
  - Both matmuls in fp16 (1 col/cycle PE; f32r would be 1/4 rate).
    fp16 energy error ~0.06 -> rel err ~7e-3, well under the 2e-2 gate.
  - +x residual folded into matmul2 via A'' = gamma*P^T + diag(Z);
    out = diag(1/Z) * (A''^T @ x) with the same Z making the x term exact.
  - PE stream order per iter: [mm1(b,m0), mm1(b,m1)] then interleaved
    [T(b,m0), mm2(b-1,m0), T(b,m1), mm2(b-1,m1)] so the softmax chain
    (DVE reduce + ACT exp) for each half hides under the other half's
    matmul and the PE never stalls on the A'' build.
  - Batch-0 xt loads split into quarters so the first matmul starts
    ~3 us earlier; final-batch stores split 4x to drain the tail.
"""

import numpy as np

B, C, T = 32, 256, 4096
NCORES = 8
NB = B // NCORES  # batches per core
P = 128
KT = T // P  # 32 k-tiles for the energy matmul
KH = KT // 2
KQ = KT // 4
TC = T // 512  # 8 t-chunks for matmul2

_CACHE = {}


def _build_nc():
    from contextlib import ExitStack

    import concourse.bacc as bacc
    import concourse.bass as bass
    import concourse.tile as tile
    from concourse import mybir

    f32 = mybir.dt.float32
    f16 = mybir.dt.float16
    ts = bass.ts

    nc = bacc.Bacc(
        "TRN2",
        target_bir_lowering=False,
        debug=False,
        enable_asserts=False,
        num_devices=NCORES,
    )

    xt_h = nc.dram_tensor("xt", [NB, P, KT, C], f16, kind="ExternalInput")
    xn_h = nc.dram_tensor("xn", [NB, P, 2, T], f16, kind="ExternalInput")
    # aux: per-partition row [gamma, 1/gamma, pad, pad, identity-row(128)]
    aux_h = nc.dram_tensor("aux", [P, 132], f32, kind="ExternalInput")
    o_h = nc.dram_tensor("o", [NB, C, T], f16, kind="ExternalOutput")

    with tile.TileContext(nc) as tc:
        with ExitStack() as ctx:
            singles = ctx.enter_context(tc.tile_pool(name="singles", bufs=1))
            xq_pool = ctx.enter_context(tc.tile_pool(name="xq", bufs=1))
            xt_pool = ctx.enter_context(tc.tile_pool(name="xt", bufs=3))
            xn_pool = ctx.enter_context(tc.tile_pool(name="xn", bufs=3))
            out_pool = ctx.enter_context(tc.tile_pool(name="out", bufs=2))
            att_pool = ctx.enter_context(tc.tile_pool(name="att", bufs=3))
            small = ctx.enter_context(tc.tile_pool(name="small", bufs=4))
            psum_e = ctx.enter_context(
                tc.tile_pool(name="psum_e", bufs=2, space="PSUM")
            )
            psum_t = ctx.enter_context(
                tc.tile_pool(name="psum_t", bufs=2, space="PSUM")
            )
            psum_o = ctx.enter_context(
                tc.tile_pool(name="psum_o", bufs=4, space="PSUM")
            )

            xt_ap = xt_h.ap()
            xn_ap = xn_h.ap()
            o_ap = o_h.ap()

            # aux on the ACT ring so it doesn't delay the first xt load
            aux = singles.tile([P, 132], f32)
            nc.scalar.dma_start(aux[:], aux_h.ap())
            gv = aux[:, 0:1]
            ident = aux[:, 4:132]
            identf = singles.tile([P, P], f16)
            nc.vector.tensor_copy(identf[:], ident)

            def issue_loads(b):
                if b == 0:
                    # quarter tiles: first matmul starts after 512 KB lands
                    qs = []
                    for q in range(4):
                        t_ = xq_pool.tile(
                            [P, KQ, C], f16, tag=f"xq{q}", name=f"xq{q}"
                        )
                        nc.sync.dma_start(
                            t_[:], xt_ap[b, :, q * KQ : (q + 1) * KQ, :]
                        )
                        qs.append(t_)
                    xt_tiles, kdiv = qs, KQ
                else:
                    xta = xt_pool.tile([P, KH, C], f16, tag="xta", name="xta")
                    xtb = xt_pool.tile([P, KH, C], f16, tag="xtb", name="xtb")
                    nc.sync.dma_start(xta[:], xt_ap[b, :, :KH, :])
                    nc.sync.dma_start(xtb[:], xt_ap[b, :, KH:, :])
                    xt_tiles, kdiv = [xta, xtb], KH
                xn = xn_pool.tile([P, 2, T], f16, tag="xn", name="xn")
                nc.sync.dma_start(xn[:], xn_ap[b])
                return xt_tiles, kdiv, xn

            def build_At(At, Pms, Zs, m):
                for k in range(2):
                    pt = psum_t.tile([P, P], f16)
                    nc.tensor.transpose(pt[:], Pms[m][:, ts(k, P)], identf[:])
                    # A''T[j in k-block, i in m-block] = gamma * P^T
                    nc.scalar.mul(At[:, k, ts(m, P)], pt[:], gv)
                # diagonal: += diag(Z) (falls in the k == m block)
                dg = small.tile([P, P], f16, tag="diag")
                nc.vector.tensor_scalar_mul(dg[:], ident, Zs[:, m : m + 1])
                nc.vector.tensor_add(
                    At[:, m, ts(m, P)], At[:, m, ts(m, P)], dg[:]
                )

            def mm2_half(pb, pAt, prZ, pxn, m):
                ot = out_pool.tile([P, T], f16, tag="ot", name="ot")
                for t8 in range(TC):
                    po = psum_o.tile([P, 512], f32)
                    for k in range(2):
                        nc.tensor.matmul(
                            po[:],
                            lhsT=pAt[:, k, ts(m, P)],
                            rhs=pxn[:, k, ts(t8, 512)],
                            start=(k == 0),
                            stop=(k == 1),
                        )
                    # out = psum * (1/Z); alternate engines
                    if t8 % 2 == 0:
                        nc.vector.tensor_scalar_mul(
                            ot[:, ts(t8, 512)], po[:], prZ[:, m : m + 1]
                        )
                    else:
                        nc.scalar.mul(
                            ot[:, ts(t8, 512)], po[:], prZ[:, m : m + 1]
                        )
                nsplit = 4 if pb == NB - 1 else 2
                for sh in range(nsplit):
                    nc.sync.dma_start(
                        o_ap[pb].rearrange("(m p) t -> p m t", p=P)[
                            :, m, ts(sh, T // nsplit)
                        ],
                        ot[:, ts(sh, T // nsplit)],
                    )

            tiles = {0: issue_loads(0)}
            pending = None  # (b, At, rZ, xn) awaiting matmul2

            for b in range(NB):
                xt, kdiv, xn = tiles.pop(b)
                if b + 1 < NB:
                    tiles[b + 1] = issue_loads(b + 1)

                # A''^T, laid out [128(j within k-block), k-block, 256(i)]
                At = att_pool.tile([P, 2, C], f16)
                Zs = small.tile([P, 2], f32)
                Zb = small.tile([P, 2], f16)
                rZ = small.tile([P, 2], f32)

                # both mm1 halves back-to-back on the PE
                pes = []
                for m in range(2):
                    pe = psum_e.tile([P, C], f32)
                    for k in range(KT):
                        src_t = xt[k // kdiv]
                        kk = k % kdiv
                        nc.tensor.matmul(
                            pe[:],
                            lhsT=src_t[:, kk, ts(m, P)],
                            rhs=src_t[:, kk, :],
                            start=(k == 0),
                            stop=(k == KT - 1),
                        )
                    pes.append(pe)

                # softmax chains (DVE + ACT only; PE-free)
                Pms = []
                for m in range(2):
                    mu = small.tile([P, 1], f32)
                    nc.vector.tensor_reduce(
                        mu[:], pes[m][:], axis=mybir.AxisListType.X,
                        op=mybir.AluOpType.min,
                    )
                    Pm = small.tile([P, C], f16, tag=f"Pm{m}")
                    nc.scalar.activation(
                        Pm[:],
                        pes[m][:],
                        mybir.ActivationFunctionType.Exp,
                        bias=mu[:],
                        scale=-1.0,
                        accum_out=Zs[:, m : m + 1],
                    )
                    nc.vector.tensor_copy(Zb[:, m : m + 1], Zs[:, m : m + 1])
                    nc.vector.reciprocal(rZ[:, m : m + 1], Zb[:, m : m + 1])
                    Pms.append(Pm)

                # interleave A'' build (tiny PE transposes) with the
                # previous batch's matmul2 halves
                prev = pending
                pending = (b, At, rZ, xn)
                build_At(At, Pms, Zs, 0)
                if prev is not None:
                    mm2_half(prev[0], prev[1], prev[2], prev[3], 0)
                build_At(At, Pms, Zs, 1)
                if prev is not None:
                    mm2_half(prev[0], prev[1], prev[2], prev[3], 1)
                if b == NB - 1:
                    for m in range(2):
                        mm2_half(b, At, rZ, xn, m)

    nc.compile()
    return nc


def _get_nc():
    if "nc" not in _CACHE:
        _CACHE["nc"] = _build_nc()
    return _CACHE["nc"]


def _make_aux(gamma_val):
    aux = np.zeros((P, 132), dtype=np.float32)
    aux[:, 0] = gamma_val
    aux[:, 1] = 1.0 / gamma_val if gamma_val != 0 else 0.0
    aux[:, 4:132] = np.eye(P, dtype=np.float32)
    return aux


def kernel(x, gamma, _trace=False):
    import concourse.bass_utils as bass_utils

    x = np.ascontiguousarray(np.asarray(x, dtype=np.float32))
    gamma = np.asarray(gamma, dtype=np.float32).reshape(-1)

    nc = _get_nc()

    aux = _make_aux(gamma[0])
    x16 = x.astype(np.float16)
    in_maps = []
    for d in range(NCORES):
        xs = x16[d * NB : (d + 1) * NB]  # [NB, C, T]
        # xt staged: [NB, P, KT, C]; element (p, k, c) = x[c, k*P + p]
        xt = np.ascontiguousarray(
            xs.transpose(0, 2, 1).reshape(NB, KT, P, C).transpose(0, 2, 1, 3)
        )
        # xn staged: [NB, P, 2, T]; element (p, m, t) = x[m*P + p, t]
        xn = np.ascontiguousarray(
            xs.reshape(NB, 2, P, T).transpose(0, 2, 1, 3)
        )
        in_maps.append({"xt": xt, "xn": xn, "aux": aux})

    res = bass_utils.run_bass_kernel_spmd(
        nc, in_maps, core_ids=list(range(NCORES)), trace=_trace
    )
    out = np.concatenate([r["o"] for r in res.results], axis=0).astype(
        np.float32
    )
    if _trace:
        _CACHE["last_results"] = res
    return out
